# revision 6
# baseline (speedup 1.0000x reference)
"""CapsNet forward as an fp8 Bass/Tile kernel on 8 Trainium2 NeuronCores.

Same math as the bf16 baseline (uniform routing collapses the capsule stage
to a mean), but all large matmuls run in fp8e4 with DoubleRow perf mode
(2 K-tiles per pass) for the primary-caps conv and the dig projection:

  conv1:  K=81 im2col, 2 M-tiles of 128ch, fp8 (output-rate-limited)
  prim:   33 K-tiles paired into 16 DR pairs + 1 single per oy-group,
          split over 3 output-x positions (rhs AP is limited to 3 free dims)
  dig:    24 K-tiles/oy paired into 12 DR pairs, N=64

Power-of-2 scales keep everything in fp8 normal range and fold into the
host-precomputed tables; biases are all zero in the reference setup so the
per-stage relu scale bookkeeping stays exact:
  W1q=8*w1 -> C1 = 8*relu1 ; W2q=8*w2 -> psum = 64*preact2 -> U = 64*relu2
  Wdigq = dig_W * 2^17/(1152*64) -> m = psm * 2^-17 + Dbias

Per-core: 64 samples, 2 chunks of 32. im2col DMAs use one contiguous
~16.7KB descriptor per (p,q) tap row; the conv1->prim phase shuffle keeps
608B (19x*32b) runs, issued from the sync+gpsimd queues.
"""
import sys

sys.path.insert(0, "/opt/trn_rl_repo")

import numpy as np
import ml_dtypes

N_CORES = 8
B = 512
BC = B // N_CORES        # 64 samples per core
BCH = 32                 # batch chunk
NCH = BC // BCH          # 2 chunks

S1 = 8.0                 # conv1 weight scale
S2 = 8.0                 # prim weight scale
S4 = float(2 ** 17)      # dig psum scale

F8NP = ml_dtypes.float8_e4m3


def _q8(a):
    return np.clip(a, -240.0, 240.0).astype(F8NP)


# prim K-tile DR pairing: (xh, seg) tiles; seg0+seg1 same xh, seg2 x-pairs
PAIRS = [((xh, 0), (xh, 1)) for xh in range(11)] + [
    ((0, 2), (2, 2)), ((1, 2), (3, 2)), ((4, 2), (6, 2)),
    ((5, 2), (7, 2)), ((8, 2), (10, 2)),
]
SINGLE = (9, 2)


# ---------------------------------------------------------------- host tables
def _build_tables(conv1_w, conv1_b, prim_w, prim_b, dig_W, dig_Wb, out_w, out_b):
    w1 = conv1_w[:, 0].reshape(256, 81)
    w2 = prim_w[:, :, 0]                       # [co, ci, 9, 9]

    # conv1 weights: M column = T*128 + 32*dd + ci  (T=d//4, dd=d%4, c=ci*8+d)
    # DR K-split: tap = 45*u + r, taps 81..89 are zero-weight pad rows
    W1r = np.zeros((45, 2, 256), np.float32)
    Cbias = np.zeros((128, 2), np.float32)
    for c in range(256):
        ci, d = c // 8, c % 8
        T, dd = d // 4, d % 4
        mu = 32 * dd + ci
        W1r[:, 0, T * 128 + mu] = w1[c, 0:45] * S1
        W1r[0:36, 1, T * 128 + mu] = w1[c, 45:81] * S1
        Cbias[mu, T] = conv1_b[c] * S1

    def tile_w2(xh, seg):
        t = np.zeros((128, 128), np.float32)
        nphi = 4 if seg < 2 else 3
        for phi in range(nphi):
            for s in range(4):
                sy, sx = s // 2, s % 2
                p = 4 * seg + phi - 2 * sy
                q = xh - 2 * sx
                if 0 <= p <= 8 and 0 <= q <= 8:
                    for ci in range(32):
                        t[32 * phi + ci, 32 * s:32 * s + 32] = w2[:, ci, p, q] * S2
        return t

    W2stk = np.zeros((128, 17, 2, 128), np.float32)
    for j, (ta, tb) in enumerate(PAIRS):
        W2stk[:, j, 0, :] = tile_w2(*ta)
        W2stk[:, j, 1, :] = tile_w2(*tb)
    W2stk[:, 16, 0, :] = tile_w2(*SINGLE)

    Pbias = np.zeros((128, 1), np.float32)
    for s in range(4):
        Pbias[32 * s:32 * s + 32, 0] = prim_b * 64.0

    # dig tiles: t = (oy, ox, d); row 32*s+co; n = co*36 + jp*6 + ip
    Wdig = np.zeros((128, 3, 12, 2, 16), np.float32)
    dscale = S4 / (1152.0 * 64.0)
    for oy in range(3):
        for ox in range(3):
            for d in range(8):
                t = ox * 8 + d
                j, u = t // 2, t % 2
                for s in range(4):
                    sy, sx = s // 2, s % 2
                    ip, jp = 2 * oy + sy, 2 * ox + sx
                    for co in range(32):
                        n = co * 36 + jp * 6 + ip
                        Wdig[32 * s + co, oy, j, u, :] = dig_W[n, d] * dscale

    return dict(
        W1r=_q8(W1r),
        Cbias=Cbias,
        W2stk=_q8(W2stk),
        Pbias=Pbias,
        Wdig=_q8(Wdig),
        Dbias=(dig_Wb.sum(0) / 1152.0).reshape(16, 1).astype(np.float32),
        W2sT=np.ascontiguousarray(out_w[..., 0].sum(1).T).astype(np.float32),
    )


# ---------------------------------------------------------------- bass kernel
def _build_nc():
    import concourse.bacc as bacc
    import concourse.bass as bass
    import concourse.mybir as mybir
    import concourse.tile as tile
    from concourse.masks import make_identity

    f8 = mybir.dt.float8e4
    f32 = mybir.dt.float32
    AF = mybir.ActivationFunctionType
    AX = mybir.AxisListType
    DR = mybir.MatmulPerfMode.DoubleRow
    ALU = mybir.AluOpType

    nc = bacc.Bacc(None, target_bir_lowering=False)

    xT_in = nc.dram_tensor("xT", [1, NCH * 784 * BCH + 28 * BCH], f8,
                           kind="ExternalInput")
    W1r_d = nc.dram_tensor("W1r", [45, 2, 256], f8, kind="ExternalInput")
    W2stk_d = nc.dram_tensor("W2stk", [128, 17, 2, 128], f8, kind="ExternalInput")
    Wdig_d = nc.dram_tensor("Wdig", [128, 3, 12, 2, 16], f8, kind="ExternalInput")
    Cbias_d = nc.dram_tensor("Cbias", [128, 2], f32, kind="ExternalInput")
    Pbias_d = nc.dram_tensor("Pbias", [128, 1], f32, kind="ExternalInput")
    Dbias_d = nc.dram_tensor("Dbias", [16, 1], f32, kind="ExternalInput")
    W2sT_d = nc.dram_tensor("W2sT", [16, 10], f32, kind="ExternalInput")
    out_d = nc.dram_tensor("out", [BC, 10], f32, kind="ExternalOutput")

    # C1ph strides (elements): [128p, 5yg, 8d, 19x, 32b]
    SYG, SD, SX = 8 * 19 * BCH, 19 * BCH, BCH
    CPH_FREE = 5 * SYG
    # conv1 y orders: phi-grouped; chunk 0's order respects the two im2col
    # y-halves so early matmuls never wait on the second half's DMAs
    YORD0 = [3, 7, 0, 4, 8, 1, 5, 9, 2, 6, 11, 15, 12, 16, 13, 17, 14, 18, 10]
    PHI0 = {15: 3, 16: 0, 17: 1, 10: 2}
    YORD1 = [3, 7, 11, 15, 0, 4, 8, 12, 16, 1, 5, 9, 13, 17, 2, 6, 10, 14, 18]
    PHI1 = {15: 3, 16: 0, 17: 1, 18: 2}

    with tile.TileContext(nc) as tc:
        with (
            tc.tile_pool(name="consts", bufs=1) as consts,
            tc.tile_pool(name="work", bufs=2) as work,
            tc.tile_pool(name="usb", bufs=1) as usbp,
            tc.tile_pool(name="fin", bufs=1) as fin,
            tc.tile_pool(name="psc", bufs=2, space="PSUM") as pscp,
            tc.tile_pool(name="pspr", bufs=2, space="PSUM") as psprp,
            tc.tile_pool(name="psm", bufs=1, space="PSUM") as psmp,
            nc.allow_non_contiguous_dma("im2col/shuffle gathers are strided"),
        ):
            # ---- input-critical constants first (weights load after im2col(0))
            W1sb = consts.tile([45, 2, 256], f8)
            nc.sync.dma_start(out=W1sb, in_=W1r_d[:, :, :])
            Cb = consts.tile([128, 2], f32)
            nc.scalar.dma_start(out=Cb, in_=Cbias_d[:, :])

            U_sb = usbp.tile([128, 3, 3, 8, BC], f8)   # [p, oy, ox, d, b]
            Uv = U_sb.rearrange("p o x d b -> p o (x d) b")
            evict_i = [0]

            def evict(out, in_, bias):
                if evict_i[0] % 2 == 0:
                    nc.scalar.activation(out=out, in_=in_, func=AF.Relu,
                                         bias=bias, scale=1.0)
                else:
                    nc.vector.tensor_scalar(out=out, in0=in_, scalar1=bias,
                                            scalar2=0.0, op0=ALU.add, op1=ALU.max)
                evict_i[0] += 1

            def emit_im2col(k, ysplits):
                # out rows are full 28-wide (x>=19 junk never read); the input
                # run reads the same length, over-reading into the next rows
                # (real data, zero-padded at the tensor tail)
                im1 = work.tile([45, 2, 19, 28, BCH], f8, tag="im1", name="im1")
                engs = [nc.sync, nc.gpsimd, nc.scalar]
                ei = 0
                for (y0, y1) in ysplits:
                    run = (y1 - y0) * 28 * BCH
                    for u in range(2):
                        for pp in range(5):
                            engs[ei % 3].dma_start(
                                out=im1[9 * pp:9 * pp + 9, u, y0:y1, :, :],
                                in_=bass.AP(
                                    tensor=xT_in,
                                    offset=(k * 784 + 28 * (5 * u + pp)
                                            + 28 * y0) * BCH,
                                    ap=[[BCH, 9], [1, run]]),
                            )
                            ei += 1
                return im1

            def emit_conv1(k, im1):
                C1t = [work.tile([128, 19, 19, BCH], f8, tag=f"c1t{T}",
                                 name=f"c1t{T}") for T in range(2)]
                C1ph = work.tile([128, 5, 8, 19, BCH], f8, tag="c1ph", name="c1ph")

                IMF = 2 * 19 * 28 * BCH
                yorder = YORD0 if k == 0 else YORD1
                phimap = PHI0 if k == 0 else PHI1

                def conv1_steps():
                    for T in range(2):
                        for y in yorder:
                            psc = pscp.tile([128, 19, BCH], f32, tag="psc",
                                            name="psc")
                            rhsA = bass.AP(tensor=im1.tensor, offset=y * 28 * BCH,
                                           ap=[[IMF, 45], [19 * 28 * BCH, 2],
                                               [1, 16 * BCH]])
                            rhsB = bass.AP(tensor=im1.tensor,
                                           offset=(y * 28 + 16) * BCH,
                                           ap=[[IMF, 45], [19 * 28 * BCH, 2],
                                               [1, 3 * BCH]])
                            nc.tensor.matmul(psc[:, 0:16, :],
                                             W1sb[:, :, 128 * T:128 * (T + 1)],
                                             rhsA, start=True, stop=True,
                                             perf_mode=DR)
                            nc.tensor.matmul(psc[:, 16:19, :],
                                             W1sb[:, :, 128 * T:128 * (T + 1)],
                                             rhsB, start=True, stop=True,
                                             perf_mode=DR)
                            # evict x-halves on both engines in parallel
                            nc.scalar.activation(out=C1t[T][:, y, 0:10, :],
                                                 in_=psc[:, 0:10, :],
                                                 func=AF.Relu,
                                                 bias=Cb[:, T:T + 1], scale=1.0)
                            nc.vector.tensor_scalar(
                                out=C1t[T][:, y, 10:19, :],
                                in0=psc[:, 10:19, :],
                                scalar1=Cb[:, T:T + 1], scalar2=0.0,
                                op0=ALU.add, op1=ALU.max)
                            phi = phimap.get(y)
                            if phi is not None:
                                ny = len(range(phi, 19, 4))
                                for dd in range(4):
                                    eng = nc.sync if dd % 2 == 0 else nc.gpsimd
                                    eng.dma_start(
                                        out=C1ph[32 * phi:32 * phi + 32, 0:ny,
                                                 4 * T + dd, :, :],
                                        in_=C1t[T][32 * dd:32 * dd + 32,
                                                   phi::4, :, :],
                                    )
                            yield
                return C1ph, conv1_steps()

            def emit_prim_ox(k, C1ph, oy, ox):
                # one (oy, ox) output group: 16 DR pairs + 1 single, N=256
                ps = psprp.tile([128, 8, BCH], f32, tag="pspr", name="pspr")
                for j, (ta, tb) in enumerate(PAIRS):
                    xa, sega = ta
                    xb, segb = tb
                    kt_stride = ((segb - sega) * SYG + (xb - xa) * SX)
                    rows = 128 if sega < 2 else 96
                    rhs = bass.AP(
                        tensor=C1ph.tensor,
                        offset=(oy + sega) * SYG + (xa + 4 * ox) * SX,
                        ap=[[CPH_FREE, rows], [kt_stride, 2], [SD, 8],
                            [1, BCH]])
                    nc.tensor.matmul(ps, W2sb[0:rows, j, :, :], rhs,
                                     start=(j == 0), stop=False, perf_mode=DR)
                rhs1 = bass.AP(tensor=C1ph.tensor,
                               offset=(2 + oy) * SYG + (SINGLE[0] + 4 * ox) * SX,
                               ap=[[CPH_FREE, 96], [SD, 8], [1, BCH]])
                nc.tensor.matmul(ps, W2sb[0:96, 16, 0, :], rhs1,
                                 start=False, stop=True)
                evict(U_sb[:, oy, ox, :, k * BCH:(k + 1) * BCH], ps, Pb[:, 0:1])

            psm = psmp.tile([16, BC], f32, tag="psm")

            def emit_dig_oy(oy):
                for j in range(12):
                    t = oy * 12 + j
                    nc.tensor.matmul(psm, Wdsb[:, oy, j, :, :],
                                     Uv[:, oy, 2 * j:2 * j + 2, :],
                                     start=(t == 0), stop=(t == 35), perf_mode=DR)

            # ---- schedule (2 chunks); input chunk 0 first, weights behind it
            im1_0 = emit_im2col(0, [(0, 10), (10, 19)])

            W2sb = consts.tile([128, 17, 2, 128], f8)
            for qq in range(4):
                eng = [nc.scalar, nc.gpsimd][qq % 2]
                eng.dma_start(out=W2sb[32 * qq:32 * qq + 32],
                              in_=W2stk_d[32 * qq:32 * qq + 32, :, :, :])
            Wdsb = consts.tile([128, 3, 12, 2, 16], f8)
            nc.scalar.dma_start(out=Wdsb, in_=Wdig_d[:, :, :, :, :])
            Pb = consts.tile([128, 1], f32)
            nc.scalar.dma_start(out=Pb, in_=Pbias_d[:, :])
            Db = consts.tile([16, 1], f32)
            nc.scalar.dma_start(out=Db, in_=Dbias_d[:, :])
            W2s = consts.tile([16, 10], f32)
            nc.scalar.dma_start(out=W2s, in_=W2sT_d[:, :])
            idf = consts.tile([16, 16], f32)
            make_identity(nc, idf)

            im1_1 = emit_im2col(1, [(0, 19)])

            C1ph0, steps0 = emit_conv1(0, im1_0)
            for _ in steps0:
                pass
            C1ph1, steps1 = emit_conv1(1, im1_1)
            # interleave chunk-1 conv1 (38 y-steps) with chunk-0 prim (9 groups)
            for oy in range(3):
                for ox in range(3):
                    for _ in range(4):
                        next(steps1, None)
                    emit_prim_ox(0, C1ph0, oy, ox)
            for _ in steps1:
                pass
            for oy in range(3):
                for ox in range(3):
                    emit_prim_ox(1, C1ph1, oy, ox)
                if oy >= 1:
                    emit_dig_oy(oy - 1)
            emit_dig_oy(2)

            # ---- m, squash, logits, softmax
            m_sb = fin.tile([16, BC], f32)
            nc.vector.tensor_scalar(out=m_sb, in0=psm, scalar1=1.0 / S4,
                                    scalar2=Db[:, 0:1], op0=ALU.mult, op1=ALU.add)
            psT = psmp.tile([BC, 16], f32, tag="pssm")
            nc.tensor.transpose(psT, m_sb, idf)
            mT = fin.tile([BC, 16], f32)
            nc.vector.tensor_copy(out=mT, in_=psT)
            sq = fin.tile([BC, 16], f32)
            nc.vector.tensor_mul(sq, mT, mT)
            l2 = fin.tile([BC, 1], f32)
            nc.vector.reduce_sum(l2, sq, axis=AX.X)
            nc.scalar.activation(out=l2, in_=l2, func=AF.Sqrt)
            l1 = fin.tile([BC, 1], f32)
            nc.vector.tensor_reduce(l1, mT, axis=AX.X, op=ALU.add,
                                    apply_absolute_value=True)
            den = fin.tile([BC, 1], f32)
            nc.vector.tensor_scalar(out=den, in0=l2, scalar1=1.0, scalar2=l1[:, 0:1],
                                    op0=ALU.add, op1=ALU.mult)
            rden = fin.tile([BC, 1], f32)
            nc.vector.reciprocal(rden, den)
            scl = fin.tile([BC, 1], f32)
            nc.vector.tensor_mul(scl, l2, rden)
            pslg = psmp.tile([BC, 10], f32, tag="pssm")
            nc.tensor.matmul(pslg, m_sb, W2s, start=True, stop=True)
            lg = fin.tile([BC, 10], f32)
            nc.vector.tensor_scalar(out=lg, in0=pslg, scalar1=scl[:, 0:1],
                                    scalar2=0.0, op0=ALU.mult, op1=ALU.add)
            ex = fin.tile([BC, 10], f32)
            nc.scalar.activation(out=ex, in_=lg, func=AF.Exp)
            sm = fin.tile([BC, 1], f32)
            nc.vector.reduce_sum(sm, ex, axis=AX.X)
            rsm = fin.tile([BC, 1], f32)
            nc.vector.reciprocal(rsm, sm)
            outt = fin.tile([BC, 10], f32)
            nc.vector.tensor_scalar(out=outt, in0=ex, scalar1=rsm[:, 0:1],
                                    scalar2=0.0, op0=ALU.mult, op1=ALU.add)
            nc.sync.dma_start(out=out_d[:, :], in_=outt)

    nc.finalize()
    return nc


_CACHE = {}


def kernel(**inputs):
    from concourse.bass_utils import run_bass_kernel_spmd

    np_in = {k: np.asarray(v) for k, v in inputs.items()}
    tabs = _build_tables(
        np_in["conv1_w"], np_in["conv1_b"], np_in["prim_w"], np_in["prim_b"],
        np_in["dig_W"], np_in["dig_Wb"], np_in["out_w"], np_in["out_b"],
    )
    x = np_in["x"][:, 0].reshape(B, 784).astype(np.float32)
    xTs = []
    for c in range(N_CORES):
        xc = x[c * BC:(c + 1) * BC]                        # [64, 784]
        flat = np.ascontiguousarray(
            xc.reshape(NCH, BCH, 784).transpose(0, 2, 1)).reshape(-1)
        arr = np.zeros((1, NCH * 784 * BCH + 28 * BCH), F8NP)
        arr[0, :flat.size] = flat.astype(F8NP)
        xTs.append(arr)

    if "nc" not in _CACHE:
        _CACHE["nc"] = _build_nc()
    nc = _CACHE["nc"]

    shared = {
        "W1r": tabs["W1r"], "W2stk": tabs["W2stk"], "Wdig": tabs["Wdig"],
        "Cbias": tabs["Cbias"], "Pbias": tabs["Pbias"], "Dbias": tabs["Dbias"],
        "W2sT": tabs["W2sT"],
    }
    in_maps = [dict(shared, xT=xTs[c]) for c in range(N_CORES)]
    res = run_bass_kernel_spmd(nc, in_maps, core_ids=list(range(N_CORES)),
                               **_CACHE.get("run_kwargs", {}))
    _CACHE["last_result"] = res
    out = np.concatenate([res.results[c]["out"] for c in range(N_CORES)], axis=0)
    return out.astype(np.float32)


# revision 7
# speedup vs baseline: 1.0074x; 1.0074x over previous
"""CapsNet forward as an fp8 Bass/Tile kernel on 8 Trainium2 NeuronCores.

Same math as the bf16 baseline (uniform routing collapses the capsule stage
to a mean), but all large matmuls run in fp8e4 with DoubleRow perf mode
(2 K-tiles per pass) for the primary-caps conv and the dig projection:

  conv1:  K=81 im2col, 2 M-tiles of 128ch, fp8 (output-rate-limited)
  prim:   33 K-tiles paired into 16 DR pairs + 1 single per oy-group,
          split over 3 output-x positions (rhs AP is limited to 3 free dims)
  dig:    24 K-tiles/oy paired into 12 DR pairs, N=64

Power-of-2 scales keep everything in fp8 normal range and fold into the
host-precomputed tables; biases are all zero in the reference setup so the
per-stage relu scale bookkeeping stays exact:
  W1q=8*w1 -> C1 = 8*relu1 ; W2q=8*w2 -> psum = 64*preact2 -> U = 64*relu2
  Wdigq = dig_W * 2^17/(1152*64) -> m = psm * 2^-17 + Dbias

Per-core: 64 samples, 2 chunks of 32. im2col DMAs use one contiguous
~16.7KB descriptor per (p,q) tap row; the conv1->prim phase shuffle keeps
608B (19x*32b) runs, issued from the sync+gpsimd queues.
"""
import sys

sys.path.insert(0, "/opt/trn_rl_repo")

import numpy as np
import ml_dtypes

N_CORES = 8
B = 512
BC = B // N_CORES        # 64 samples per core
BCH = 32                 # batch chunk
NCH = BC // BCH          # 2 chunks

S1 = 8.0                 # conv1 weight scale
S2 = 8.0                 # prim weight scale
S4 = float(2 ** 17)      # dig psum scale

F8NP = ml_dtypes.float8_e4m3


def _q8(a):
    return np.clip(a, -240.0, 240.0).astype(F8NP)


# prim K-tile DR pairing: (xh, seg) tiles; seg0+seg1 same xh, seg2 x-pairs
PAIRS = [((xh, 0), (xh, 1)) for xh in range(11)] + [
    ((0, 2), (2, 2)), ((1, 2), (3, 2)), ((4, 2), (6, 2)),
    ((5, 2), (7, 2)), ((8, 2), (10, 2)),
]
SINGLE = (9, 2)


# ---------------------------------------------------------------- host tables
def _build_tables(conv1_w, conv1_b, prim_w, prim_b, dig_W, dig_Wb, out_w, out_b):
    w1 = conv1_w[:, 0].reshape(256, 81)
    w2 = prim_w[:, :, 0]                       # [co, ci, 9, 9]

    # conv1 weights: M column = T*128 + 32*dd + ci  (T=d//4, dd=d%4, c=ci*8+d)
    # DR K-split: tap = 45*u + r, taps 81..89 are zero-weight pad rows
    W1r = np.zeros((45, 2, 256), np.float32)
    Cbias = np.zeros((128, 2), np.float32)
    for c in range(256):
        ci, d = c // 8, c % 8
        T, dd = d // 4, d % 4
        mu = 32 * dd + ci
        W1r[:, 0, T * 128 + mu] = w1[c, 0:45] * S1
        W1r[0:36, 1, T * 128 + mu] = w1[c, 45:81] * S1
        Cbias[mu, T] = conv1_b[c] * S1

    def tile_w2(xh, seg):
        t = np.zeros((128, 128), np.float32)
        nphi = 4 if seg < 2 else 3
        for phi in range(nphi):
            for s in range(4):
                sy, sx = s // 2, s % 2
                p = 4 * seg + phi - 2 * sy
                q = xh - 2 * sx
                if 0 <= p <= 8 and 0 <= q <= 8:
                    for ci in range(32):
                        t[32 * phi + ci, 32 * s:32 * s + 32] = w2[:, ci, p, q] * S2
        return t

    W2stk = np.zeros((128, 17, 2, 128), np.float32)
    for j, (ta, tb) in enumerate(PAIRS):
        W2stk[:, j, 0, :] = tile_w2(*ta)
        W2stk[:, j, 1, :] = tile_w2(*tb)
    W2stk[:, 16, 0, :] = tile_w2(*SINGLE)

    Pbias = np.zeros((128, 1), np.float32)
    for s in range(4):
        Pbias[32 * s:32 * s + 32, 0] = prim_b * 64.0

    # dig tiles: t = (oy, ox, d); row 32*s+co; n = co*36 + jp*6 + ip
    Wdig = np.zeros((128, 3, 12, 2, 16), np.float32)
    dscale = S4 / (1152.0 * 64.0)
    for oy in range(3):
        for ox in range(3):
            for d in range(8):
                t = ox * 8 + d
                j, u = t // 2, t % 2
                for s in range(4):
                    sy, sx = s // 2, s % 2
                    ip, jp = 2 * oy + sy, 2 * ox + sx
                    for co in range(32):
                        n = co * 36 + jp * 6 + ip
                        Wdig[32 * s + co, oy, j, u, :] = dig_W[n, d] * dscale

    return dict(
        W1r=_q8(W1r),
        Cbias=Cbias,
        W2stk=_q8(W2stk),
        Pbias=Pbias,
        Wdig=_q8(Wdig),
        Dbias=(dig_Wb.sum(0) / 1152.0).reshape(16, 1).astype(np.float32),
        W2sT=np.ascontiguousarray(out_w[..., 0].sum(1).T).astype(np.float32),
    )


# ---------------------------------------------------------------- bass kernel
def _build_nc():
    import concourse.bacc as bacc
    import concourse.bass as bass
    import concourse.mybir as mybir
    import concourse.tile as tile
    from concourse.masks import make_identity

    f8 = mybir.dt.float8e4
    f32 = mybir.dt.float32
    AF = mybir.ActivationFunctionType
    AX = mybir.AxisListType
    DR = mybir.MatmulPerfMode.DoubleRow
    ALU = mybir.AluOpType

    nc = bacc.Bacc(None, target_bir_lowering=False)

    xT_in = nc.dram_tensor("xT", [1, NCH * 784 * BCH + 28 * BCH], f8,
                           kind="ExternalInput")
    W1r_d = nc.dram_tensor("W1r", [45, 2, 256], f8, kind="ExternalInput")
    W2stk_d = nc.dram_tensor("W2stk", [128, 17, 2, 128], f8, kind="ExternalInput")
    Wdig_d = nc.dram_tensor("Wdig", [128, 3, 12, 2, 16], f8, kind="ExternalInput")
    Cbias_d = nc.dram_tensor("Cbias", [128, 2], f32, kind="ExternalInput")
    Pbias_d = nc.dram_tensor("Pbias", [128, 1], f32, kind="ExternalInput")
    Dbias_d = nc.dram_tensor("Dbias", [16, 1], f32, kind="ExternalInput")
    W2sT_d = nc.dram_tensor("W2sT", [16, 10], f32, kind="ExternalInput")
    out_d = nc.dram_tensor("out", [BC, 10], f32, kind="ExternalOutput")

    # C1ph strides (elements): [128p, 5yg, 8d, 19x, 32b]
    SYG, SD, SX = 8 * 19 * BCH, 19 * BCH, BCH
    CPH_FREE = 5 * SYG
    # conv1 y orders: phi-grouped; chunk 0's order respects the two im2col
    # y-halves so early matmuls never wait on the second half's DMAs
    YORD0 = [3, 7, 0, 4, 8, 1, 5, 9, 2, 6, 11, 15, 12, 16, 13, 17, 14, 18, 10]
    PHI0 = {15: 3, 16: 0, 17: 1, 10: 2}
    YORD1 = [3, 7, 11, 15, 0, 4, 8, 12, 16, 1, 5, 9, 13, 17, 2, 6, 10, 14, 18]
    PHI1 = {15: 3, 16: 0, 17: 1, 18: 2}

    with tile.TileContext(nc) as tc:
        with (
            tc.tile_pool(name="consts", bufs=1) as consts,
            tc.tile_pool(name="work", bufs=2) as work,
            tc.tile_pool(name="usb", bufs=1) as usbp,
            tc.tile_pool(name="fin", bufs=1) as fin,
            tc.tile_pool(name="psc", bufs=2, space="PSUM") as pscp,
            tc.tile_pool(name="pspr", bufs=2, space="PSUM") as psprp,
            tc.tile_pool(name="psm", bufs=1, space="PSUM") as psmp,
            nc.allow_non_contiguous_dma("im2col/shuffle gathers are strided"),
        ):
            # ---- input-critical constants first (weights load after im2col(0))
            W1sb = consts.tile([45, 2, 256], f8)
            nc.sync.dma_start(out=W1sb, in_=W1r_d[:, :, :])
            Cb = consts.tile([128, 2], f32)
            nc.scalar.dma_start(out=Cb, in_=Cbias_d[:, :])

            U_sb = usbp.tile([128, 3, 3, 8, BC], f8)   # [p, oy, ox, d, b]
            Uv = U_sb.rearrange("p o x d b -> p o (x d) b")
            evict_i = [0]

            def evict(out, in_, bias):
                if evict_i[0] % 2 == 0:
                    nc.scalar.activation(out=out, in_=in_, func=AF.Relu,
                                         bias=bias, scale=1.0)
                else:
                    nc.vector.tensor_scalar(out=out, in0=in_, scalar1=bias,
                                            scalar2=0.0, op0=ALU.add, op1=ALU.max)
                evict_i[0] += 1

            def emit_im2col(k, ysplits):
                # out rows are full 28-wide (x>=19 junk never read); the input
                # run reads the same length, over-reading into the next rows
                # (real data, zero-padded at the tensor tail)
                im1 = work.tile([45, 2, 19, 28, BCH], f8, tag="im1", name="im1")
                IMF = 2 * 19 * 28 * BCH
                engs = [nc.sync, nc.gpsimd, nc.scalar]
                ei = 0
                for (y0, y1) in ysplits:
                    run = (y1 - y0) * 28 * BCH
                    for u in range(2):
                        for pp in range(5):
                            engs[ei % 3].dma_start(
                                out=bass.AP(
                                    tensor=im1.tensor,
                                    offset=(9 * pp) * IMF
                                    + (u * 19 + y0) * 28 * BCH,
                                    ap=[[IMF, 9], [1, run]]),
                                in_=bass.AP(
                                    tensor=xT_in,
                                    offset=(k * 784 + 28 * (5 * u + pp)
                                            + 28 * y0) * BCH,
                                    ap=[[BCH, 9], [1, run]]),
                            )
                            ei += 1
                return im1

            def emit_conv1(k, im1):
                C1t = [work.tile([128, 19, 19, BCH], f8, tag=f"c1t{T}",
                                 name=f"c1t{T}") for T in range(2)]
                C1ph = work.tile([128, 5, 8, 19, BCH], f8, tag="c1ph", name="c1ph")

                IMF = 2 * 19 * 28 * BCH
                yorder = YORD0 if k == 0 else YORD1
                phimap = PHI0 if k == 0 else PHI1

                def conv1_steps():
                    for T in range(2):
                        for y in yorder:
                            psc = pscp.tile([128, 19, BCH], f32, tag="psc",
                                            name="psc")
                            rhsA = bass.AP(tensor=im1.tensor, offset=y * 28 * BCH,
                                           ap=[[IMF, 45], [19 * 28 * BCH, 2],
                                               [1, 16 * BCH]])
                            rhsB = bass.AP(tensor=im1.tensor,
                                           offset=(y * 28 + 16) * BCH,
                                           ap=[[IMF, 45], [19 * 28 * BCH, 2],
                                               [1, 3 * BCH]])
                            nc.tensor.matmul(psc[:, 0:16, :],
                                             W1sb[:, :, 128 * T:128 * (T + 1)],
                                             rhsA, start=True, stop=True,
                                             perf_mode=DR)
                            nc.tensor.matmul(psc[:, 16:19, :],
                                             W1sb[:, :, 128 * T:128 * (T + 1)],
                                             rhsB, start=True, stop=True,
                                             perf_mode=DR)
                            # evict x-halves on both engines in parallel
                            nc.scalar.activation(out=C1t[T][:, y, 0:10, :],
                                                 in_=psc[:, 0:10, :],
                                                 func=AF.Relu,
                                                 bias=Cb[:, T:T + 1], scale=1.0)
                            nc.vector.tensor_scalar(
                                out=C1t[T][:, y, 10:19, :],
                                in0=psc[:, 10:19, :],
                                scalar1=Cb[:, T:T + 1], scalar2=0.0,
                                op0=ALU.add, op1=ALU.max)
                            phi = phimap.get(y)
                            if phi is not None:
                                ny = len(range(phi, 19, 4))
                                for dd in range(4):
                                    eng = nc.sync if dd % 2 == 0 else nc.gpsimd
                                    eng.dma_start(
                                        out=C1ph[32 * phi:32 * phi + 32, 0:ny,
                                                 4 * T + dd, :, :],
                                        in_=C1t[T][32 * dd:32 * dd + 32,
                                                   phi::4, :, :],
                                    )
                            yield
                return C1ph, conv1_steps()

            def emit_prim_ox(k, C1ph, oy, ox):
                # one (oy, ox) output group: 16 DR pairs + 1 single, N=256
                ps = psprp.tile([128, 8, BCH], f32, tag="pspr", name="pspr")
                for j, (ta, tb) in enumerate(PAIRS):
                    xa, sega = ta
                    xb, segb = tb
                    kt_stride = ((segb - sega) * SYG + (xb - xa) * SX)
                    rows = 128 if sega < 2 else 96
                    rhs = bass.AP(
                        tensor=C1ph.tensor,
                        offset=(oy + sega) * SYG + (xa + 4 * ox) * SX,
                        ap=[[CPH_FREE, rows], [kt_stride, 2], [SD, 8],
                            [1, BCH]])
                    nc.tensor.matmul(ps, W2sb[0:rows, j, :, :], rhs,
                                     start=(j == 0), stop=False, perf_mode=DR)
                rhs1 = bass.AP(tensor=C1ph.tensor,
                               offset=(2 + oy) * SYG + (SINGLE[0] + 4 * ox) * SX,
                               ap=[[CPH_FREE, 96], [SD, 8], [1, BCH]])
                nc.tensor.matmul(ps, W2sb[0:96, 16, 0, :], rhs1,
                                 start=False, stop=True)
                evict(U_sb[:, oy, ox, :, k * BCH:(k + 1) * BCH], ps, Pb[:, 0:1])

            psm = psmp.tile([16, BC], f32, tag="psm")

            def emit_dig_oy(oy):
                for j in range(12):
                    t = oy * 12 + j
                    nc.tensor.matmul(psm, Wdsb[:, oy, j, :, :],
                                     Uv[:, oy, 2 * j:2 * j + 2, :],
                                     start=(t == 0), stop=(t == 35), perf_mode=DR)

            # ---- schedule (2 chunks); input chunk 0 first, weights behind it
            im1_0 = emit_im2col(0, [(0, 10), (10, 19)])

            W2sb = consts.tile([128, 17, 2, 128], f8)
            for qq in range(4):
                eng = [nc.scalar, nc.gpsimd][qq % 2]
                eng.dma_start(out=W2sb[32 * qq:32 * qq + 32],
                              in_=W2stk_d[32 * qq:32 * qq + 32, :, :, :])
            Wdsb = consts.tile([128, 3, 12, 2, 16], f8)
            nc.scalar.dma_start(out=Wdsb, in_=Wdig_d[:, :, :, :, :])
            Pb = consts.tile([128, 1], f32)
            nc.scalar.dma_start(out=Pb, in_=Pbias_d[:, :])
            Db = consts.tile([16, 1], f32)
            nc.scalar.dma_start(out=Db, in_=Dbias_d[:, :])
            W2s = consts.tile([16, 10], f32)
            nc.scalar.dma_start(out=W2s, in_=W2sT_d[:, :])
            idf = consts.tile([16, 16], f32)
            make_identity(nc, idf)

            im1_1 = emit_im2col(1, [(0, 19)])

            C1ph0, steps0 = emit_conv1(0, im1_0)
            for _ in steps0:
                pass
            C1ph1, steps1 = emit_conv1(1, im1_1)
            # interleave chunk-1 conv1 (38 y-steps) with chunk-0 prim (9 groups)
            for oy in range(3):
                for ox in range(3):
                    for _ in range(4):
                        next(steps1, None)
                    emit_prim_ox(0, C1ph0, oy, ox)
            for _ in steps1:
                pass
            for oy in range(3):
                for ox in range(3):
                    emit_prim_ox(1, C1ph1, oy, ox)
                if oy >= 1:
                    emit_dig_oy(oy - 1)
            emit_dig_oy(2)

            # ---- m, squash, logits, softmax
            m_sb = fin.tile([16, BC], f32)
            nc.vector.tensor_scalar(out=m_sb, in0=psm, scalar1=1.0 / S4,
                                    scalar2=Db[:, 0:1], op0=ALU.mult, op1=ALU.add)
            psT = psmp.tile([BC, 16], f32, tag="pssm")
            nc.tensor.transpose(psT, m_sb, idf)
            mT = fin.tile([BC, 16], f32)
            nc.vector.tensor_copy(out=mT, in_=psT)
            sq = fin.tile([BC, 16], f32)
            nc.vector.tensor_mul(sq, mT, mT)
            l2 = fin.tile([BC, 1], f32)
            nc.vector.reduce_sum(l2, sq, axis=AX.X)
            nc.scalar.activation(out=l2, in_=l2, func=AF.Sqrt)
            l1 = fin.tile([BC, 1], f32)
            nc.vector.tensor_reduce(l1, mT, axis=AX.X, op=ALU.add,
                                    apply_absolute_value=True)
            den = fin.tile([BC, 1], f32)
            nc.vector.tensor_scalar(out=den, in0=l2, scalar1=1.0, scalar2=l1[:, 0:1],
                                    op0=ALU.add, op1=ALU.mult)
            rden = fin.tile([BC, 1], f32)
            nc.vector.reciprocal(rden, den)
            scl = fin.tile([BC, 1], f32)
            nc.vector.tensor_mul(scl, l2, rden)
            pslg = psmp.tile([BC, 10], f32, tag="pssm")
            nc.tensor.matmul(pslg, m_sb, W2s, start=True, stop=True)
            lg = fin.tile([BC, 10], f32)
            nc.vector.tensor_scalar(out=lg, in0=pslg, scalar1=scl[:, 0:1],
                                    scalar2=0.0, op0=ALU.mult, op1=ALU.add)
            ex = fin.tile([BC, 10], f32)
            nc.scalar.activation(out=ex, in_=lg, func=AF.Exp)
            sm = fin.tile([BC, 1], f32)
            nc.vector.reduce_sum(sm, ex, axis=AX.X)
            rsm = fin.tile([BC, 1], f32)
            nc.vector.reciprocal(rsm, sm)
            outt = fin.tile([BC, 10], f32)
            nc.vector.tensor_scalar(out=outt, in0=ex, scalar1=rsm[:, 0:1],
                                    scalar2=0.0, op0=ALU.mult, op1=ALU.add)
            nc.sync.dma_start(out=out_d[:, :], in_=outt)

    nc.finalize()
    return nc


_CACHE = {}


def kernel(**inputs):
    from concourse.bass_utils import run_bass_kernel_spmd

    np_in = {k: np.asarray(v) for k, v in inputs.items()}
    tabs = _build_tables(
        np_in["conv1_w"], np_in["conv1_b"], np_in["prim_w"], np_in["prim_b"],
        np_in["dig_W"], np_in["dig_Wb"], np_in["out_w"], np_in["out_b"],
    )
    x = np_in["x"][:, 0].reshape(B, 784).astype(np.float32)
    xTs = []
    for c in range(N_CORES):
        xc = x[c * BC:(c + 1) * BC]                        # [64, 784]
        flat = np.ascontiguousarray(
            xc.reshape(NCH, BCH, 784).transpose(0, 2, 1)).reshape(-1)
        arr = np.zeros((1, NCH * 784 * BCH + 28 * BCH), F8NP)
        arr[0, :flat.size] = flat.astype(F8NP)
        xTs.append(arr)

    if "nc" not in _CACHE:
        _CACHE["nc"] = _build_nc()
    nc = _CACHE["nc"]

    shared = {
        "W1r": tabs["W1r"], "W2stk": tabs["W2stk"], "Wdig": tabs["Wdig"],
        "Cbias": tabs["Cbias"], "Pbias": tabs["Pbias"], "Dbias": tabs["Dbias"],
        "W2sT": tabs["W2sT"],
    }
    in_maps = [dict(shared, xT=xTs[c]) for c in range(N_CORES)]
    res = run_bass_kernel_spmd(nc, in_maps, core_ids=list(range(N_CORES)),
                               **_CACHE.get("run_kwargs", {}))
    _CACHE["last_result"] = res
    out = np.concatenate([res.results[c]["out"] for c in range(N_CORES)], axis=0)
    return out.astype(np.float32)
